# revision 1
# baseline (speedup 1.0000x reference)
"""AttentionHead kernel for 8x TRN2 NeuronCores (Bass/Tile on Bacc).

Problem: single-head attention, S=4096, B=4, D=128, C=K=V=64, f32 inputs,
int32 {0,1} mask [1, S, S] applied before softmax (mask==0 -> -inf).

Sharding: queries sharded across 8 cores (512 q/core, all 4 batches per
core). Host passes everything pre-laid-out in bf16: key/query/value
feature-major [D, B, S*], mask slice pre-transposed [S, QS]. Per-core
HBM read ~12.6 MiB.

Math (per core, per batch), all PE contractions on partitions:
  k_proj = wk @ key              (NO bias: softmax over s is invariant
                                  to the per-q offset bk.(q+bq))
  q_proj = wq @ query + bq
  v_projb[s,c] = value[s,:] @ wv[c,:] + bv[c]     (projected UP FRONT)
  v_ext[s, 0:64] = v_projb;  v_ext[s, 64] = 1     (ones column)
  scores^T[s, q] = sum_c k_proj[s,c] q_proj[q,c]  (even/odd row-split)
  alpha = exp(scores^T / 8) * maskT               (ACT exp, DVE mult)
  comb[c', q] = sum_s v_ext[s,c'] alpha[s,q]      (M=65 matmul: row 64
                                                   accumulates the softmax
                                                   denominator for free --
                                                   no separate sums matmuls)
  out[q, :] = (comb[0:64]/comb[64]).T             (PE transpose + scale;
                                                   the bv*sums term divides
                                                   out to exactly +bv)

v_ext construction: v_projT [c, s] via wvT-stationary matmuls (c on
partitions), bias added during the PSUM->SBUF copy (per-partition bias),
ones as literal row 64 of an [80, S] staging tile, then HWDGE xbar
DMA-transpose [80, 512] -> [128, 4, 80] slots (contiguous destination;
cols 65..79 of each slot are dead padding). va2 lhsT = v_ext[:, 80t:80t+65].

Perf structure:
  - staged software pipeline: iteration u issues scores(u), exp+mask(u-1),
    va2(u-2). PE never waits on the scores->exp->mask->alpha chain, so it
    stays continuously busy (required to reach/hold the high PE p-state).
  - one full-pair exp [128,1024] per iteration (per-instruction overhead
    dominates small ACT/DVE ops; bigger is better).
  - ACT: exp + projection-copy/bias; DVE: mask mult + epilogue; PE: 3.2
    matmul issues per pair instead of 5 (sums matmuls eliminated).
  - host-side bf16 removes all f32->bf16 CAST traffic and halves DMA.
  - next batch's key/query/value DMAs are emitted before this batch's
    main loop so the transfers hide under compute.
"""

import os
import sys

import numpy as np

if "/opt/trn_rl_repo" not in sys.path:
    sys.path.insert(0, "/opt/trn_rl_repo")

S, B, D, C = 4096, 4, 128, 64
NCORES = 8
QS = S // NCORES  # 512 queries per core
QT = QS // 128  # 4 q tiles
ST = S // 128  # 32 s tiles
NP = ST // 2  # 16 even/odd s-tile pairs
KEXP = 1  # exp/mask lag behind scores (pairs)
KVA = 4  # va lag behind scores (pairs): generous slack so va2
         # never races the exp/mask chain or the v_ext transposes
SLOT = 80  # v_ext slot width (64 proj + 1 ones + 15 pad; 160B = 32B-aligned)
SCALE = 0.125  # 1/sqrt(64)

LAST_RESULT = None
KVER = 37  # bumped per kernel revision: defeats HLO-fingerprint NEFF-cache aliasing


def _install_ntff_hook():
    """The grading/axon image lacks antenv.axon_hooks; recreate it so
    trace=True can capture NTFF profiles. Harmless no-op when unavailable."""
    import types

    try:
        import antenv

        try:
            from antenv import axon_hooks  # noqa: F401

            return
        except ImportError:
            pass
        from trn_agent_boot.trn_boot import _ntff_profile_via_ctypes

        mod = types.ModuleType("antenv.axon_hooks")
        _h = [_ntff_profile_via_ctypes("/opt/axon/libaxon_pjrt.so")]
        mod.get_axon_ntff_profile_hook = lambda: _h[0]
        mod.set_axon_ntff_profile_hook = lambda h: _h.__setitem__(0, h)
        sys.modules["antenv.axon_hooks"] = mod
        antenv.axon_hooks = mod
    except Exception:
        pass


def _build_nc():
    import concourse.mybir as mybir
    from concourse import bacc
    from concourse.masks import make_identity
    from concourse.tile import TileContext

    f32 = mybir.dt.float32
    bf16 = mybir.dt.bfloat16
    AF = mybir.ActivationFunctionType

    nc = bacc.Bacc("TRN2")

    key_d = nc.dram_tensor("key", [D, B, S], bf16, kind="ExternalInput")
    query_d = nc.dram_tensor("query", [D, B, QS], bf16, kind="ExternalInput")
    value_d = nc.dram_tensor("value", [D, B, S], bf16, kind="ExternalInput")
    mask_d = nc.dram_tensor("mask", [S, QS], bf16, kind="ExternalInput")
    w2T_d = nc.dram_tensor("w2T", [D, D], bf16, kind="ExternalInput")
    wvT_d = nc.dram_tensor("wvT", [D, SLOT], bf16, kind="ExternalInput")
    b2_d = nc.dram_tensor("b2", [D, 1], f32, kind="ExternalInput")
    bvx_d = nc.dram_tensor("bvx", [SLOT, 1], f32, kind="ExternalInput")
    out_d = nc.dram_tensor("out", [QS, B, C], f32, kind="ExternalOutput")
    # dummy input whose shape encodes the kernel revision: the PJRT-side NEFF
    # cache keys on the HLO signature (not the embedded BIR), so same-shaped
    # kernel revisions would otherwise silently alias to a stale executable.
    nc.dram_tensor("vtag", [KVER], f32, kind="ExternalInput")

    with TileContext(nc) as tc:
        with (
            tc.tile_pool(name="consts", bufs=1) as consts,
            tc.tile_pool(name="big", bufs=1) as big,
            tc.tile_pool(name="pb", bufs=2) as pb,
            tc.tile_pool(name="work", bufs=4) as work,
            tc.tile_pool(name="expool", bufs=3) as expool,
            tc.tile_pool(name="apool", bufs=6) as apool,
            tc.tile_pool(name="scps", bufs=3, space="PSUM") as scps,
            tc.tile_pool(name="ppps", bufs=1, space="PSUM") as ppps,
            tc.tile_pool(name="accps", bufs=1, space="PSUM") as accps,
        ):
            # ---------------- constants ----------------
            ident_f = consts.tile([128, 128], f32, tag="ident_f")
            make_identity(nc, ident_f[:])

            # weights arrive pre-transposed bf16 from the host; wvT is
            # pre-padded to [D, 80] (zero cols 64-79) and bvx pre-extended
            # (row 64 = 1.0 -> the ones row of v_projbT, rows 65-79 = 0).
            wT = {}
            for name, d_t, cols in (("2", w2T_d, D), ("v", wvT_d, SLOT)):
                wt_sb = consts.tile([D, cols], bf16, name=f"wt_sb_{name}")
                nc.sync.dma_start(out=wt_sb[:], in_=d_t[:, :])
                wT[name] = wt_sb
            b2 = consts.tile([D, 1], f32, tag="b2")
            nc.sync.dma_start(out=b2[:], in_=b2_d[:, :])
            bv1x = consts.tile([SLOT, 1], f32, tag="bv1x")
            nc.sync.dma_start(out=bv1x[:], in_=bvx_d[:, :])

            maskT = big.tile([128, ST * QS], bf16, tag="maskT")

            def load_batch(b):
                keyT = pb.tile([128, S], bf16, tag="keyT")
                qT = pb.tile([128, QS], bf16, tag="qT")
                nc.sync.dma_start(out=qT[:], in_=query_d[:, b, :])
                for h in range(2):
                    nc.sync.dma_start(
                        out=keyT[:, h * 2048 : (h + 1) * 2048],
                        in_=key_d[:, b, h * 2048 : (h + 1) * 2048],
                    )
                valT = pb.tile([128, S], bf16, tag="valT")
                for h in range(2):
                    nc.sync.dma_start(
                        out=valT[:, h * 2048 : (h + 1) * 2048],
                        in_=value_d[:, b, h * 2048 : (h + 1) * 2048],
                    )
                return keyT, qT, valT

            def proj_tiles():
                qq = pb.tile([128, QS], bf16, tag="qq")
                v_projbT = pb.tile([SLOT, S], bf16, tag="v_projbT")
                v_ext = pb.tile([128, ST * SLOT], bf16, tag="v_ext")
                return qq, v_ext, v_projbT

            def kq_tasks(tiles, keyT, qT):
                """Query-side projection: wk is folded into W2 = wk^T wq on
                the host, so scores need only qq = W2 q + b2 (one matmul)."""
                qq, v_ext, v_projbT = tiles
                cell = {}

                def qq_mm():
                    qq_ps = ppps.tile([128, 512], f32, tag="pp", name="qq_ps")
                    nc.tensor.matmul(
                        qq_ps[:], wT["2"][:], qT[:], start=True, stop=True
                    )
                    cell["ps"] = qq_ps

                def qq_cp():
                    nc.scalar.activation(
                        out=qq[:],
                        in_=cell["ps"][:],
                        func=AF.Identity,
                        bias=b2[:],
                        scale=1.0,
                    )

                return [(qq_mm, qq_cp)]

            def v_tasks(tiles, valT):
                """Value-side (mm, copy) phase pairs + transpose steps."""
                qq, v_ext, v_projbT = tiles
                pairs = []
                for i in range(8):
                    cell = {}

                    def vp_mm(i=i, cell=cell):
                        vp_ps = ppps.tile([SLOT, 512], f32, tag="pp", name="vp_ps")
                        nc.tensor.matmul(
                            vp_ps[:],
                            wT["v"][:],
                            valT[:, i * 512 : (i + 1) * 512],
                            start=True,
                            stop=True,
                        )
                        cell["ps"] = vp_ps

                    def vp_cp(i=i, cell=cell):
                        dst = v_projbT[:, i * 512 : (i + 1) * 512]
                        if i % 4 == 0:
                            nc.scalar.activation(
                                out=dst,
                                in_=cell["ps"][:],
                                func=AF.Identity,
                                bias=bv1x[:],
                                scale=1.0,
                            )
                        else:
                            nc.vector.tensor_scalar_add(
                                out=dst, in0=cell["ps"][:], scalar1=bv1x[:]
                            )

                    pairs.append((vp_mm, vp_cp))
                    if i % 2 == 1:

                        def tr_task(i=i):
                            j = i // 2
                            nc.sync.dma_start_transpose(
                                out=v_ext[
                                    :, j * 8 * SLOT : (j + 1) * 8 * SLOT
                                ].rearrange("p (tt c) -> p tt c", c=SLOT),
                                in_=v_projbT[:, j * 1024 : (j + 1) * 1024],
                            )

                        pairs.append((tr_task, None))
                return pairs

            def stagger(pairs):
                """Turn (mm, copy) pairs into per-iteration steps where each
                step emits the PREVIOUS task's copy before this task's mm, so
                the single-buffer pp ring never stalls the PE in-order queue."""
                steps = []
                prev_cp = [None]

                def mk(mm, pc):
                    def step():
                        if pc is not None:
                            pc()
                        mm()

                    return step

                for mm, cp in pairs:
                    steps.append(mk(mm, prev_cp[0]))
                    prev_cp[0] = cp
                if prev_cp[0] is not None:
                    steps.append(lambda pc=prev_cp[0]: pc())
                return steps

            def epilogue_tasks(b, va2_ps):
                """Deferred epilogue steps, drained inside the NEXT batch's
                loop so the batch boundary never idles PE/ACT (idle gaps
                re-throttle the PE clock). Step 0 (comb copy) frees va2_ps
                and must run early; the rest can trail."""
                cell = {}
                steps = []

                def comb_step():
                    comb = work.tile([C + 1, QS], f32, tag="comb")
                    nc.vector.tensor_copy(out=comb[:], in_=va2_ps[:])
                    fin = work.tile([128, QT * C], f32, tag="fin")
                    cell["comb"] = comb
                    cell["fin"] = fin

                steps.append(comb_step)
                for qt in range(QT):

                    def qt_step(qt=qt):
                        ot_ps = ppps.tile([128, C + 1], f32, tag="pp", name="ot_ps")
                        nc.tensor.transpose(
                            ot_ps[:],
                            cell["comb"][:, qt * 128 : (qt + 1) * 128],
                            ident_f[: C + 1, : C + 1],
                        )
                        recip = work.tile([128, 1], f32, tag="recip")
                        nc.vector.reciprocal(recip[:], ot_ps[:, C : C + 1])
                        nc.vector.tensor_scalar_mul(
                            out=cell["fin"][:, qt * C : (qt + 1) * C],
                            in0=ot_ps[:, :C],
                            scalar1=recip[:],
                        )

                    steps.append(qt_step)

                def out_step():
                    nc.sync.dma_start(
                        out=out_d[:, b, :].rearrange("(qt p) c -> p qt c", p=128),
                        in_=cell["fin"][:].rearrange("p (qt c) -> p qt c", c=C),
                    )

                steps.append(out_step)
                return steps

            # ---------- flat pipeline over all B*NP pairs ----------
            # Batch b+1's scores start during batch b's va2 drain, so no
            # engine idles at a batch seam (idle re-throttles the PE clock).
            loaded = load_batch(0)
            keyT_by_batch = {0: loaded[0]}
            tiles_by_batch = {0: proj_tiles()}
            for mm, cp in kq_tasks(tiles_by_batch[0], loaded[0], loaded[1]):
                mm()
                if cp is not None:
                    cp()
            pending = stagger(v_tasks(tiles_by_batch[0], loaded[2]))
            for j in range(8):
                nc.sync.dma_start(
                    out=maskT[:, j * 2048 : (j + 1) * 2048].rearrange(
                        "p (t q) -> p t q", t=4
                    ),
                    in_=mask_d[j * 512 : (j + 1) * 512, :].rearrange(
                        "(t p) q -> p t q", p=128
                    ),
                )

            total = B * NP
            va2_by_batch = {}
            scs = {}
            alphas = {}
            for up in range(total + KVA):
                if up < total:
                    b_s, u_s = divmod(up, NP)
                    if u_s == 0 and b_s + 1 < B:
                        nxt_loaded = load_batch(b_s + 1)
                        keyT_by_batch[b_s + 1] = nxt_loaded[0]
                        nxt_tiles = proj_tiles()
                        tiles_by_batch[b_s + 1] = nxt_tiles
                        pending += stagger(
                            kq_tasks(nxt_tiles, nxt_loaded[0], nxt_loaded[1])
                            + v_tasks(nxt_tiles, nxt_loaded[2])
                        )
                    qq = tiles_by_batch[b_s][0]
                    keyT = keyT_by_batch[b_s]
                    sc = scps.tile([128, 1024], f32, tag="sc", name="sc")
                    for h in range(2):
                        st = 2 * u_s + h
                        nc.tensor.matmul(
                            sc[:, h * 512 : (h + 1) * 512],
                            keyT[:, st * 128 : (st + 1) * 128],
                            qq[:],
                            start=True,
                            stop=True,
                        )
                    scs[up] = sc
                if KEXP <= up < total + KEXP:
                    v = up - KEXP
                    u_e = v % NP
                    sc = scs.pop(v)
                    ex = expool.tile([128, 1024], bf16, tag="ex")
                    nc.scalar.activation(
                        out=ex[:], in_=sc[:], func=AF.Exp, scale=SCALE
                    )
                    alpha = apool.tile([128, 1024], bf16, tag="alpha")
                    nc.vector.tensor_mul(
                        alpha[:], ex[:], maskT[:, u_e * 1024 : (u_e + 1) * 1024]
                    )
                    alphas[v] = alpha
                if up >= KVA:
                    v = up - KVA
                    if v < total:
                        b_v, u_v = divmod(v, NP)
                        if u_v == 0:
                            va2_by_batch[b_v] = accps.tile(
                                [C + 1, QS], f32, tag="va", name="va2"
                            )
                        va2_ps = va2_by_batch[b_v]
                        v_ext = tiles_by_batch[b_v][1]
                        alpha = alphas.pop(v)
                        for h in range(2):
                            st = 2 * u_v + h
                            nc.tensor.matmul(
                                va2_ps[:],
                                v_ext[:, st * SLOT : st * SLOT + C + 1],
                                alpha[:, h * 512 : (h + 1) * 512],
                                start=(st == 0),
                                stop=(st == ST - 1),
                            )
                        if u_v == NP - 1:
                            ep = epilogue_tasks(b_v, va2_by_batch.pop(b_v))
                            pending[:0] = [ep[0]]
                            pending.extend(ep[1:])
                            del tiles_by_batch[b_v]
                if pending:
                    pending.pop(0)()
                if len(pending) > 14:
                    pending.pop(0)()
            while pending:
                pending.pop(0)()

    nc.finalize()
    return nc


_nc_cache = None


def kernel(**inputs):
    global _nc_cache, LAST_RESULT
    _install_ntff_hook()
    import ml_dtypes

    from concourse.bass_utils import run_bass_kernel_spmd

    bf16 = ml_dtypes.bfloat16
    arrs = {k: np.asarray(v) for k, v in inputs.items()}
    key = np.ascontiguousarray(
        arrs["key"].astype(np.float32).transpose(2, 1, 0)
    ).astype(bf16)
    value = np.ascontiguousarray(
        arrs["value"].astype(np.float32).transpose(2, 1, 0)
    ).astype(bf16)
    query = np.ascontiguousarray(arrs["query"], dtype=np.float32)
    mask = np.ascontiguousarray(arrs["mask"], dtype=np.int32)
    if mask.ndim == 3:
        mask = mask[0]

    wk_f = arrs["wk_w"].astype(np.float64)
    wq_f = arrs["wq_w"].astype(np.float64)
    # scores = (wk key).(wq q + bq) = key^T (W2 q + b2); lhsT for the qq
    # matmul is W2^T = wq^T wk
    w2T = np.ascontiguousarray(wq_f.T @ wk_f).astype(np.float32).astype(bf16)
    b2h = np.ascontiguousarray(
        (wk_f.T @ arrs["wq_b"].astype(np.float64)).astype(np.float32)
    ).reshape(D, 1)
    wvT = np.zeros([D, SLOT], dtype=bf16)
    wvT[:, :C] = arrs["wv_w"].astype(np.float32).T.astype(bf16)
    bvx = np.zeros([SLOT, 1], np.float32)
    bvx[:C, 0] = arrs["wv_b"].astype(np.float32)
    bvx[C, 0] = 1.0

    if _nc_cache is None:
        _nc_cache = _build_nc()
    nc = _nc_cache

    in_maps = []
    for i in range(NCORES):
        q0 = i * QS
        in_maps.append(
            {
                "key": key,
                "value": value,
                "query": np.ascontiguousarray(
                    query[q0 : q0 + QS].transpose(2, 1, 0)
                ).astype(bf16),
                "mask": np.ascontiguousarray(mask[q0 : q0 + QS].T).astype(bf16),
                "w2T": w2T,
                "wvT": wvT,
                "b2": b2h,
                "bvx": bvx,
                "vtag": np.zeros([KVER], np.float32),
            }
        )

    trace = bool(int(os.environ.get("KERNEL_TRACE", "0")))
    kw = {}
    if trace:
        kw = dict(trace=True, trace_cores=[0])
    try:
        res = run_bass_kernel_spmd(nc, in_maps, core_ids=list(range(NCORES)), **kw)
    except Exception:
        # transient device wedge (e.g. NRT_EXEC_UNIT_UNRECOVERABLE from an
        # earlier crashed process): one retry after the runtime re-opens
        res = run_bass_kernel_spmd(nc, in_maps, core_ids=list(range(NCORES)), **kw)
    LAST_RESULT = res
    out = np.concatenate([r["out"] for r in res.results], axis=0)
    return out



# revision 7
# speedup vs baseline: 1.3194x; 1.3194x over previous
"""AttentionHead kernel for 8x TRN2 NeuronCores (Bass/Tile on Bacc).

Problem: single-head attention, S=4096, B=4, D=128, C=K=V=64, f32 inputs,
int32 {0,1} mask [1, S, S] applied before softmax (mask==0 -> -inf).

Sharding: queries sharded across 8 cores (512 q/core, all 4 batches per
core).

Math (per core, per batch), all PE contractions on partitions:
  qq = W2 q + b2 where W2 = wk^T wq (host-folded; per-q bias bk.q is
       softmax-invariant and dropped), cast fp8e4.
  scores^T[s, q] = sum_d key8[d,s] qq8[d,q]  +  maskbias[s, q]
       computed as ONE fp8 DoubleRow matmul per 128-s-tile chunk:
       lhsT [128, 2, 128] = [identity | key8_tile]   (host-interleaved)
       rhs  [128, 2, 512] = [mask8_chunk | qq8]      (custom-stride AP over
            one SBUF arena [mask(16K) | qq_b0..b3]; j=0 identity-delivers
            maskbias = -240*(1-mask) exactly, j=1 is the real k^T q)
  alpha = exp(scores/8)  (ACT, [128, 1536] groups, writes fp8 directly;
       masked entries exp(~-28) underflow to exactly 0)
  v_ext[s, c'] = fp8(value_tile^T wv), c'=64 column = 1 (memset), built
       directly in [s, c'] orientation (no transposes); bias bv deferred.
  comb[c', q] += v_ext_pair^T alpha_pair   (fp8 DoubleRow, K=256: two
       s-tiles per matmul; row 64 accumulates the softmax denominator)
  out[q, :] = comb[0:64]/comb[64] + bv     (PE transpose + fused
       affine_then_add: *recip + bv, bv host-replicated [128, 64])

Perf structure: ACT exp (11 instrs x ~1.55us per batch) is the bottleneck
engine; PE (scores 32 + va2 16 + vproj 32 + qq/epi per batch) runs ~50us
busy with slack, DVE ~15us, DMA ~8.5 MiB/core. Deep software pipeline:
group g scores || g-1 exp || lagged va2 pairs || staggered vproj/qq/epilogue
pending tasks keep every engine fed across batch seams.
"""

import os
import sys

import numpy as np

if "/opt/trn_rl_repo" not in sys.path:
    sys.path.insert(0, "/opt/trn_rl_repo")

S, B, D, C = 4096, 4, 128, 64
NCORES = 8
QS = S // NCORES  # 512 queries per core
QT = QS // 128  # 4 q tiles
ST = S // 128  # 32 s tiles per batch
NG = 11  # exp groups per batch: 10x(3 chunks) + 1x(2 chunks)
NP = ST // 2  # 16 va2 pairs per batch
SLOT = 128  # v_ext slot stride in elements (64 proj + 1 ones + pad;
# LDWEIGHTS DoubleRow requires well-aligned j-plane strides — 68 fails
# the walrus ISA check, 128 is the micro-proven shape)
MASKW = ST * QS  # 16384 arena mask columns
SCALE = 0.125  # 1/sqrt(64)
MASKBIAS = -240.0  # exact in fp8e4m3; exp(scale*(x-240)) == 0 for |x|<~100
ALPHA_FP8 = True  # False: bf16 alpha + non-DR va2 (higher precision)

LAST_RESULT = None
KVER = 40  # bumped per kernel revision: defeats HLO-fingerprint NEFF-cache aliasing


def _install_ntff_hook():
    """The grading/axon image lacks antenv.axon_hooks; recreate it so
    trace=True can capture NTFF profiles. Harmless no-op when unavailable."""
    import types

    try:
        import antenv

        try:
            from antenv import axon_hooks  # noqa: F401

            return
        except ImportError:
            pass
        from trn_agent_boot.trn_boot import _ntff_profile_via_ctypes

        mod = types.ModuleType("antenv.axon_hooks")
        _h = [_ntff_profile_via_ctypes("/opt/axon/libaxon_pjrt.so")]
        mod.get_axon_ntff_profile_hook = lambda: _h[0]
        mod.set_axon_ntff_profile_hook = lambda h: _h.__setitem__(0, h)
        sys.modules["antenv.axon_hooks"] = mod
        antenv.axon_hooks = mod
    except Exception:
        pass


def _gwidth(g):
    """chunks in group g (local index)"""
    return 3 if g < 10 else 2


def _gcol(g):
    """first chunk index of group g"""
    return 3 * g


def _build_nc():
    import concourse.mybir as mybir
    from concourse import bacc
    from concourse.masks import make_identity
    from concourse.tile import TileContext

    f32 = mybir.dt.float32
    bf16 = mybir.dt.bfloat16
    f8 = mybir.dt.float8e4
    AF = mybir.ActivationFunctionType
    DR = mybir.MatmulPerfMode.DoubleRow
    a_dt = f8 if ALPHA_FP8 else bf16

    nc = bacc.Bacc("TRN2")

    key8_d = nc.dram_tensor("key8", [D, B, ST * 256], f8, kind="ExternalInput")
    query_d = nc.dram_tensor("query", [D, B, QS], bf16, kind="ExternalInput")
    val_d = nc.dram_tensor("val", [D, B, S], bf16, kind="ExternalInput")
    mask8_d = nc.dram_tensor("mask8", [S, QS], f8, kind="ExternalInput")
    w2T_d = nc.dram_tensor("w2T", [D, D], bf16, kind="ExternalInput")
    wvT_d = nc.dram_tensor("wvT", [D, C], bf16, kind="ExternalInput")
    b2_d = nc.dram_tensor("b2", [D, 1], f32, kind="ExternalInput")
    bvrep_d = nc.dram_tensor("bvrep", [128, C], f32, kind="ExternalInput")
    out_d = nc.dram_tensor("out", [QS, B, C], f32, kind="ExternalOutput")
    # dummy input whose shape encodes the kernel revision: the PJRT-side NEFF
    # cache keys on the HLO signature (not the embedded BIR), so same-shaped
    # kernel revisions would otherwise silently alias to a stale executable.
    nc.dram_tensor("vtag", [KVER], f32, kind="ExternalInput")

    with TileContext(nc) as tc:
        with (
            tc.tile_pool(name="consts", bufs=1) as consts,
            tc.tile_pool(name="big", bufs=1) as big,
            tc.tile_pool(name="pb", bufs=2) as pb,
            tc.tile_pool(name="apool", bufs=2) as apool,
            tc.tile_pool(name="work", bufs=4) as work,
            tc.tile_pool(name="scps", bufs=2, space="PSUM") as scps,
            tc.tile_pool(name="ppps", bufs=1, space="PSUM") as ppps,
            tc.tile_pool(name="accps", bufs=1, space="PSUM") as accps,
        ):
            # ---------------- constants ----------------
            ident_f = consts.tile([128, 128], f32, tag="ident_f")
            make_identity(nc, ident_f[:])

            w2T = consts.tile([D, D], bf16, tag="w2T")
            nc.sync.dma_start(out=w2T[:], in_=w2T_d[:, :])
            wvT = consts.tile([D, C], bf16, tag="wvT")
            nc.sync.dma_start(out=wvT[:], in_=wvT_d[:, :])
            b2 = consts.tile([D, 1], f32, tag="b2")
            nc.sync.dma_start(out=b2[:], in_=b2_d[:, :])
            bvrep = consts.tile([128, C], f32, tag="bvrep")
            nc.sync.dma_start(out=bvrep[:], in_=bvrep_d[:, :])

            # arena: [mask (MASKW) | qq_b0 | qq_b1 | qq_b2 | qq_b3] fp8
            arena = big.tile([128, MASKW + B * QS], f8, tag="arena")
            for j in range(8):
                nc.sync.dma_start(
                    out=arena[:, j * 2048 : (j + 1) * 2048].rearrange(
                        "p (t q) -> p t q", t=4
                    ),
                    in_=mask8_d[j * 512 : (j + 1) * 512, :].rearrange(
                        "(t p) q -> p t q", p=128
                    ),
                )

            def scores_rhs(b, m):
                """custom AP [128, 2, 512]: j=0 -> mask chunk m, j=1 -> qq_b"""
                base = arena[:, m * QS : (m + 1) * QS]
                ap = base.unsqueeze(1)
                l = ap.ap
                l[1] = [MASKW + b * QS - m * QS, 2]
                ap.ap = l
                return ap

            def load_batch(b):
                qT = pb.tile([128, QS], bf16, tag="qT")
                nc.sync.dma_start(out=qT[:], in_=query_d[:, b, :])
                key8 = pb.tile([128, ST * 256], f8, tag="key8")
                for h in range(2):
                    nc.sync.dma_start(
                        out=key8[:, h * 4096 : (h + 1) * 4096],
                        in_=key8_d[:, b, h * 4096 : (h + 1) * 4096],
                    )
                valT = pb.tile([128, S], bf16, tag="valT")
                for h in range(2):
                    nc.sync.dma_start(
                        out=valT[:, h * 2048 : (h + 1) * 2048],
                        in_=val_d[:, b, h * 2048 : (h + 1) * 2048],
                    )
                return qT, key8, valT

            def qq_tasks(b, qT):
                cell = {}

                def qq_mm():
                    qq_ps = ppps.tile([128, QS], f32, tag="pp", name="qq_ps")
                    nc.tensor.matmul(qq_ps[:], w2T[:], qT[:], start=True, stop=True)
                    cell["ps"] = qq_ps

                def qq_cp():
                    nc.vector.tensor_scalar_add(
                        out=arena[:, MASKW + b * QS : MASKW + (b + 1) * QS],
                        in0=cell["ps"][:],
                        scalar1=b2[:],
                    )

                return [(qq_mm, qq_cp)]

            def v_tasks(valT, v_ext):
                """Direct-orientation vproj: out[s, c] tiles, batched copies."""
                pairs = []
                # ones column: c'=64 of each slot
                pairs.append(
                    (
                        lambda: nc.vector.memset(
                            v_ext[:].rearrange("p (t c) -> p t c", c=SLOT)[
                                :, :, C : C + 1
                            ],
                            1.0,
                        ),
                        None,
                    )
                )
                # first group small so its copy lands (in program order)
                # before the first va2 pair enters the PE queue
                bounds = [0, 2, 8, 14, 20, 26, 32]
                for gi in range(len(bounds) - 1):
                    g0, g1 = bounds[gi], bounds[gi + 1]
                    gs = g1 - g0
                    cell = {}
                    for k in range(gs):

                        def vp_mm(k=k, g0=g0, cell=cell, first=(k == 0)):
                            if first:
                                cell["ps"] = ppps.tile(
                                    [128, 7 * C], f32, tag="pp", name="vp_ps"
                                )
                            st = g0 + k
                            nc.tensor.matmul(
                                cell["ps"][:, k * C : (k + 1) * C],
                                valT[:, st * 128 : (st + 1) * 128],
                                wvT[:],
                                start=True,
                                stop=True,
                            )

                        pairs.append((vp_mm, None))

                    def vp_cp(g0=g0, gs=gs, cell=cell):
                        nc.vector.tensor_copy(
                            out=v_ext[:, g0 * SLOT : (g0 + gs) * SLOT].rearrange(
                                "p (t c) -> p t c", c=SLOT
                            )[:, :, :C],
                            in_=cell["ps"][:, : gs * C].rearrange(
                                "p (t c) -> p t c", c=C
                            ),
                        )

                    pairs.append((None, vp_cp))
                return pairs

            def stagger(pairs):
                """Each step emits the PREVIOUS task's copy before this task's
                mm so the single-buffer pp ring never stalls the PE queue."""
                steps = []
                prev_cp = [None]

                def mk(mm, pc):
                    def step():
                        if pc is not None:
                            pc()
                        if mm is not None:
                            mm()

                    return step

                for mm, cp in pairs:
                    steps.append(mk(mm, prev_cp[0]))
                    prev_cp[0] = cp
                if prev_cp[0] is not None:
                    steps.append(lambda pc=prev_cp[0]: pc())
                return steps

            def epilogue_tasks(b, acc_ps):
                cell = {}
                steps = []

                def comb_step():
                    comb = work.tile([C + 1, QS], f32, tag="comb")
                    nc.vector.tensor_copy(out=comb[:], in_=acc_ps[:])
                    fin = work.tile([128, QT * C], f32, tag="fin")
                    cell["comb"] = comb
                    cell["fin"] = fin

                steps.append(comb_step)
                for qt in range(QT):

                    def qt_step(qt=qt):
                        ot_ps = ppps.tile([128, C + 1], f32, tag="pp", name="ot_ps")
                        nc.tensor.transpose(
                            ot_ps[:],
                            cell["comb"][:, qt * 128 : (qt + 1) * 128],
                            ident_f[: C + 1, : C + 1],
                        )
                        recip = work.tile([128, 1], f32, tag="recip")
                        nc.vector.reciprocal(recip[:], ot_ps[:, C : C + 1])
                        nc.vector.affine_then_add(
                            out=cell["fin"][:, qt * C : (qt + 1) * C],
                            in0=ot_ps[:, :C],
                            in1=bvrep[:],
                            scale=recip[:],
                            bias=0.0,
                        )

                    steps.append(qt_step)

                def out_step():
                    nc.sync.dma_start(
                        out=out_d[:, b, :].rearrange("(qt p) c -> p qt c", p=128),
                        in_=cell["fin"][:].rearrange("p (qt c) -> p qt c", c=C),
                    )

                steps.append(out_step)
                return steps

            # ---------- flat pipeline over all B*NG groups ----------
            loaded = load_batch(0)
            key8_by_b = {0: loaded[1]}
            vext_by_b = {}
            alpha_by_b = {}
            for mm, cp in qq_tasks(0, loaded[0]):
                mm()
                if cp is not None:
                    cp()
            v_ext0 = pb.tile([128, ST * SLOT], a_dt, tag="v_ext")
            vext_by_b[0] = v_ext0
            pending = stagger(v_tasks(loaded[2], v_ext0))

            total = B * NG
            KEXP = 1  # exp lags scores by 1 group
            scs = {}
            acc_by_b = {}
            va_cursor = 0  # global va2 pair index
            exp_done = -1  # last global group exp'd

            def issue_va2(limit_pairs):
                """Issue ready va2 pairs up to global pair index `limit_pairs`
                (exclusive)."""
                nonlocal va_cursor
                while va_cursor < min(limit_pairs, B * NP):
                    v = va_cursor
                    b_v, u = divmod(v, NP)
                    if u == 0:
                        acc_by_b[b_v] = accps.tile(
                            [C + 1, QS], f32, tag="acc", name="acc"
                        )
                    acc = acc_by_b[b_v]
                    v_ext = vext_by_b[b_v]
                    alpha = alpha_by_b[b_v]
                    if ALPHA_FP8:
                        lhsT = v_ext[:, u * 2 * SLOT : (u + 1) * 2 * SLOT].rearrange(
                            "p (j c) -> p j c", c=SLOT
                        )[:, :, : C + 1]
                        rhs = alpha[:, u * 1024 : (u + 1) * 1024].rearrange(
                            "p (j q) -> p j q", j=2
                        )
                        nc.tensor.matmul(
                            acc[:],
                            lhsT,
                            rhs,
                            start=(u == 0),
                            stop=(u == NP - 1),
                            perf_mode=mybir.MatmulPerfMode.DoubleRow,
                        )
                    else:
                        for h in range(2):
                            st = 2 * u + h
                            nc.tensor.matmul(
                                acc[:],
                                v_ext[:, st * SLOT : st * SLOT + C + 1],
                                alpha[:, st * 512 : (st + 1) * 512],
                                start=(st == 0),
                                stop=(st == ST - 1),
                            )
                    va_cursor += 1
                    if u == NP - 1:
                        ep = epilogue_tasks(b_v, acc_by_b.pop(b_v))
                        pending[:0] = [ep[0]]
                        pending.extend(ep[1:])
                        del vext_by_b[b_v]
                        del alpha_by_b[b_v]

            for gp in range(total + KEXP + 2):
                if gp < total:
                    b_s, g_s = divmod(gp, NG)
                    if g_s == 0:
                        alpha_by_b[b_s] = apool.tile(
                            [128, ST * 512], a_dt, tag="alpha", name="alpha"
                        )
                        if b_s + 1 < B:
                            nxt = load_batch(b_s + 1)
                            key8_by_b[b_s + 1] = nxt[1]
                            v_ext_n = pb.tile([128, ST * SLOT], a_dt, tag="v_ext")
                            vext_by_b[b_s + 1] = v_ext_n
                            pending.extend(
                                stagger(
                                    qq_tasks(b_s + 1, nxt[0])
                                    + v_tasks(nxt[2], v_ext_n)
                                )
                            )
                    key8 = key8_by_b[b_s]
                    w = _gwidth(g_s)
                    sc = scps.tile([128, 1536], f32, tag="sc", name="sc")
                    for ci in range(w):
                        m = _gcol(g_s) + ci
                        lhsT = key8[:, m * 256 : (m + 1) * 256].rearrange(
                            "p (j s) -> p j s", j=2
                        )
                        nc.tensor.matmul(
                            sc[:, ci * 512 : (ci + 1) * 512],
                            lhsT,
                            scores_rhs(b_s, m),
                            start=True,
                            stop=True,
                            perf_mode=DR,
                        )
                    scs[gp] = (sc, b_s, g_s, w)
                if KEXP <= gp < total + KEXP:
                    v = gp - KEXP
                    sc, b_e, g_e, w = scs.pop(v)
                    alpha = alpha_by_b[b_e]
                    c0 = _gcol(g_e) * 512
                    nc.scalar.activation(
                        out=alpha[:, c0 : c0 + w * 512],
                        in_=sc[:, : w * 512],
                        func=AF.Exp,
                        scale=SCALE,
                    )
                    exp_done = v
                    # va2 pairs fully covered by exp'd groups (one group slack)
                    eb, eg = divmod(exp_done, NG)
                    # within batch eb: chunks done through _gcol(eg)+_gwidth(eg)
                    chunks_done = _gcol(eg) + _gwidth(eg)
                    pairs_done = chunks_done // 2
                    issue_va2(eb * NP + max(0, pairs_done - 2))
                if gp >= total + KEXP:
                    issue_va2(B * NP)
                for thresh in (0, 0, 8, 16, 24):
                    if len(pending) > thresh:
                        pending.pop(0)()
            issue_va2(B * NP)
            while pending:
                pending.pop(0)()

    nc.finalize()
    return nc


_nc_cache = None


def kernel(**inputs):
    global _nc_cache, LAST_RESULT
    _install_ntff_hook()
    import ml_dtypes

    from concourse.bass_utils import run_bass_kernel_spmd

    bf16 = ml_dtypes.bfloat16
    f8 = ml_dtypes.float8_e4m3
    arrs = {k: np.asarray(v) for k, v in inputs.items()}

    keyT = np.ascontiguousarray(
        arrs["key"].astype(np.float32).transpose(2, 1, 0)
    )  # [D, B, S]
    key8 = np.zeros([D, B, ST * 256], dtype=f8)
    eye8 = np.eye(128, dtype=np.float32).astype(f8)
    k8 = keyT.astype(f8)
    for st in range(ST):
        key8[:, :, st * 256 : st * 256 + 128] = eye8[:, None, :]
        key8[:, :, st * 256 + 128 : (st + 1) * 256] = k8[
            :, :, st * 128 : (st + 1) * 128
        ]

    value = np.ascontiguousarray(
        arrs["value"].astype(np.float32).transpose(2, 1, 0)
    ).astype(bf16)  # [D, B, S]
    query = np.ascontiguousarray(arrs["query"], dtype=np.float32)
    mask = np.ascontiguousarray(arrs["mask"], dtype=np.int32)
    if mask.ndim == 3:
        mask = mask[0]
    maskbias = np.where(mask == 0, np.float32(MASKBIAS), np.float32(0.0))
    mask8_full = maskbias.astype(f8)  # [S(q-axis? no: rows=q of reference), S]

    wk_f = arrs["wk_w"].astype(np.float64)
    wq_f = arrs["wq_w"].astype(np.float64)
    # scores = (wk key).(wq q + bq) = key^T (W2 q + b2); lhsT for the qq
    # matmul is W2^T = wq^T wk
    w2T = np.ascontiguousarray(wq_f.T @ wk_f).astype(np.float32).astype(bf16)
    b2h = np.ascontiguousarray(
        (wk_f.T @ arrs["wq_b"].astype(np.float64)).astype(np.float32)
    ).reshape(D, 1)
    wvT = np.ascontiguousarray(arrs["wv_w"].astype(np.float32).T).astype(bf16)
    bvrep = np.ascontiguousarray(
        np.broadcast_to(arrs["wv_b"].astype(np.float32)[None, :], (128, C))
    ).astype(np.float32)

    if _nc_cache is None:
        _nc_cache = _build_nc()
    nc = _nc_cache

    in_maps = []
    for i in range(NCORES):
        q0 = i * QS
        # reference: scores[b, q, s] masked by mask[q, s] -> per-core mask
        # slice rows q0:q0+QS of the q axis... but our arena layout is
        # [s-part, q]: arena[p, m*512+q] = maskbias[q0+q?? NO:
        # scores^T[s, q] masked by mask[q_global, s]: chunk m covers
        # s = m*128+p, col q: maskbias_T[s, q] = maskbias[q0+q, s]
        mslice = np.ascontiguousarray(mask8_full[q0 : q0 + QS, :].T)  # [S, QS]
        in_maps.append(
            {
                "key8": key8,
                "val": value,
                "query": np.ascontiguousarray(
                    query[q0 : q0 + QS].transpose(2, 1, 0)
                ).astype(bf16),
                "mask8": mslice,
                "w2T": w2T,
                "wvT": wvT,
                "b2": b2h,
                "bvrep": bvrep,
                "vtag": np.zeros([KVER], np.float32),
            }
        )

    trace = bool(int(os.environ.get("KERNEL_TRACE", "0")))
    kw = {}
    if trace:
        kw = dict(trace=True, trace_cores=[0])
    try:
        res = run_bass_kernel_spmd(nc, in_maps, core_ids=list(range(NCORES)), **kw)
    except Exception:
        # transient device wedge (e.g. NRT_EXEC_UNIT_UNRECOVERABLE from an
        # earlier crashed process): one retry after the runtime re-opens
        res = run_bass_kernel_spmd(nc, in_maps, core_ids=list(range(NCORES)), **kw)
    LAST_RESULT = res
    out = np.concatenate([r["out"] for r in res.results], axis=0)
    return out


# revision 15
# speedup vs baseline: 1.3706x; 1.0388x over previous
"""AttentionHead kernel for 8x TRN2 NeuronCores (Bass/Tile on Bacc).

Problem: single-head attention, S=4096, B=4, D=128, C=K=V=64, f32 inputs,
int32 {0,1} mask [1, S, S] applied before softmax (mask==0 -> -inf).

Sharding: queries sharded across 8 cores (512 q/core, all 4 batches per
core).

Math (per core, per batch), all PE contractions on partitions:
  qq = W2 q + b2 where W2 = wk^T wq (host-folded; per-q bias bk.q is
       softmax-invariant and dropped), cast fp8e4.
  scores^T[s, q] = sum_d key8[d,s] qq8[d,q]  +  maskbias[s, q]
       computed as ONE fp8 DoubleRow matmul per 128-s-tile chunk:
       lhsT [128, 2, 128] = [identity | key8_tile]   (host-interleaved)
       rhs  [128, 2, 512] = [mask8_chunk | qq8]      (custom-stride AP over
            one SBUF arena [mask(16K) | qq_b0..b3]; j=0 identity-delivers
            maskbias = -240*(1-mask) exactly, j=1 is the real k^T q)
  alpha = exp(scores/8)  (ACT, [128, 1536] groups, writes fp8 directly;
       masked entries exp(~-28) underflow to exactly 0)
  v_ext[s, c'] = fp8(value_tile^T wv), c'=64 column = 1 (memset), built
       directly in [s, c'] orientation (no transposes); bias bv deferred.
  comb[c', q] += v_ext_pair^T alpha_pair   (fp8 DoubleRow, K=256: two
       s-tiles per matmul; row 64 accumulates the softmax denominator)
  out[q, :] = comb[0:64]/comb[64] + bv     (PE transpose + fused
       affine_then_add: *recip + bv, bv host-replicated [128, 64])

Perf structure: ACT exp (11 instrs x ~1.55us per batch) is the bottleneck
engine; PE (scores 32 + va2 16 + vproj 32 + qq/epi per batch) runs ~50us
busy with slack, DVE ~15us, DMA ~8.5 MiB/core. Deep software pipeline:
group g scores || g-1 exp || lagged va2 pairs || staggered vproj/qq/epilogue
pending tasks keep every engine fed across batch seams.
"""

import os
import sys

import numpy as np

if "/opt/trn_rl_repo" not in sys.path:
    sys.path.insert(0, "/opt/trn_rl_repo")

S, B, D, C = 4096, 4, 128, 64
NCORES = 8
QS = S // NCORES  # 512 queries per core
QT = QS // 128  # 4 q tiles
ST = S // 128  # 32 s tiles per batch
NG = 11  # exp groups per batch: 10x(3 chunks) + 1x(2 chunks)
NP = ST // 2  # 16 va2 pairs per batch
SLOT = 128  # v_ext slot stride in elements (64 proj + 1 ones + pad;
# LDWEIGHTS DoubleRow requires well-aligned j-plane strides — 68 fails
# the walrus ISA check, 128 is the micro-proven shape)
MASKW = ST * QS  # 16384 arena mask columns
SCALE = 0.125  # 1/sqrt(64)
MASKBIAS = -240.0  # exact in fp8e4m3; exp(scale*(x-240)) == 0 for |x|<~100
ALPHA_FP8 = True  # False: bf16 alpha + non-DR va2 (higher precision)

LAST_RESULT = None
KVER = 41  # bumped per kernel revision: defeats HLO-fingerprint NEFF-cache aliasing


def _install_ntff_hook():
    """The grading/axon image lacks antenv.axon_hooks; recreate it so
    trace=True can capture NTFF profiles. Harmless no-op when unavailable."""
    import types

    try:
        import antenv

        try:
            from antenv import axon_hooks  # noqa: F401

            return
        except ImportError:
            pass
        from trn_agent_boot.trn_boot import _ntff_profile_via_ctypes

        mod = types.ModuleType("antenv.axon_hooks")
        _h = [_ntff_profile_via_ctypes("/opt/axon/libaxon_pjrt.so")]
        mod.get_axon_ntff_profile_hook = lambda: _h[0]
        mod.set_axon_ntff_profile_hook = lambda h: _h.__setitem__(0, h)
        sys.modules["antenv.axon_hooks"] = mod
        antenv.axon_hooks = mod
    except Exception:
        pass


def _gwidth(g):
    """chunks in group g (local index)"""
    return 3 if g < 10 else 2


def _gcol(g):
    """first chunk index of group g"""
    return 3 * g


def _build_nc():
    import concourse.mybir as mybir
    from concourse import bacc
    from concourse.masks import make_identity
    from concourse.tile import TileContext

    f32 = mybir.dt.float32
    bf16 = mybir.dt.bfloat16
    f8 = mybir.dt.float8e4
    AF = mybir.ActivationFunctionType
    DR = mybir.MatmulPerfMode.DoubleRow
    a_dt = f8 if ALPHA_FP8 else bf16

    nc = bacc.Bacc("TRN2")

    key8_d = nc.dram_tensor("key8", [D, B, ST * 256], f8, kind="ExternalInput")
    query_d = nc.dram_tensor("query", [D, B, QS], bf16, kind="ExternalInput")
    val_d = nc.dram_tensor("val", [D, B, S], bf16, kind="ExternalInput")
    # mask pre-swizzled on host to the arena layout [p, m*512+q] so each
    # DMA moves contiguous 2KB-per-partition runs (512B descriptors took
    # ~21us of serial startup otherwise)
    mask8_d = nc.dram_tensor("mask8", [128, MASKW], f8, kind="ExternalInput")
    w2T_d = nc.dram_tensor("w2T", [D, D], bf16, kind="ExternalInput")
    wvT_d = nc.dram_tensor("wvT", [D, C], bf16, kind="ExternalInput")
    b2_d = nc.dram_tensor("b2", [D, 1], f32, kind="ExternalInput")
    bvrep_d = nc.dram_tensor("bvrep", [128, C], f32, kind="ExternalInput")
    # output layout [p, (b, qt, c)]: one contiguous 1KB-per-partition DMA
    # per batch (the [q, b, c] layout needed 256B descriptors); host
    # unpacks to [QS, B, C]
    out_d = nc.dram_tensor("out", [128, B * QT * C], f32, kind="ExternalOutput")
    # dummy input whose shape encodes the kernel revision: the PJRT-side NEFF
    # cache keys on the HLO signature (not the embedded BIR), so same-shaped
    # kernel revisions would otherwise silently alias to a stale executable.
    nc.dram_tensor("vtag", [KVER], f32, kind="ExternalInput")

    with TileContext(nc) as tc:
        with (
            tc.tile_pool(name="consts", bufs=1) as consts,
            tc.tile_pool(name="big", bufs=1) as big,
            tc.tile_pool(name="pb", bufs=2) as pb,
            tc.tile_pool(name="apool", bufs=2) as apool,
            tc.tile_pool(name="work", bufs=4) as work,
            tc.tile_pool(name="scps", bufs=2, space="PSUM") as scps,
            tc.tile_pool(name="ppps", bufs=1, space="PSUM") as ppps,
            tc.tile_pool(name="accps", bufs=1, space="PSUM") as accps,
        ):
            # ---------------- constants ----------------
            ident_f = consts.tile([128, 128], f32, tag="ident_f")
            make_identity(nc, ident_f[:])

            w2T = consts.tile([D, D], bf16, tag="w2T")
            nc.sync.dma_start(out=w2T[:], in_=w2T_d[:, :])
            wvT = consts.tile([D, C], bf16, tag="wvT")
            nc.sync.dma_start(out=wvT[:], in_=wvT_d[:, :])
            b2 = consts.tile([D, 1], f32, tag="b2")
            nc.sync.dma_start(out=b2[:], in_=b2_d[:, :])
            bvrep = consts.tile([128, C], f32, tag="bvrep")
            nc.sync.dma_start(out=bvrep[:], in_=bvrep_d[:, :])

            # arena: [mask (MASKW) | qq_b0 | qq_b1 | qq_b2 | qq_b3] fp8
            arena = big.tile([128, MASKW + B * QS], f8, tag="arena")

            def load_mask():
                for j in range(8):
                    nc.sync.dma_start(
                        out=arena[:, j * 2048 : (j + 1) * 2048],
                        in_=mask8_d[:, j * 2048 : (j + 1) * 2048],
                    )

            def scores_rhs(b, m):
                """custom AP [128, 2, 512]: j=0 -> mask chunk m, j=1 -> qq_b"""
                base = arena[:, m * QS : (m + 1) * QS]
                ap = base.unsqueeze(1)
                l = ap.ap
                l[1] = [MASKW + b * QS - m * QS, 2]
                ap.ap = l
                return ap

            def load_batch(b):
                qT = pb.tile([128, QS], bf16, tag="qT")
                nc.sync.dma_start(out=qT[:], in_=query_d[:, b, :])
                key8 = pb.tile([128, ST * 256], f8, tag="key8")
                for h in range(2):
                    nc.sync.dma_start(
                        out=key8[:, h * 4096 : (h + 1) * 4096],
                        in_=key8_d[:, b, h * 4096 : (h + 1) * 4096],
                    )
                valT = pb.tile([128, S], bf16, tag="valT")
                for h in range(2):
                    nc.sync.dma_start(
                        out=valT[:, h * 2048 : (h + 1) * 2048],
                        in_=val_d[:, b, h * 2048 : (h + 1) * 2048],
                    )
                return qT, key8, valT

            def qq_tasks(b, qT):
                cell = {}

                def qq_mm():
                    qq_ps = ppps.tile([128, QS], f32, tag="pp", name="qq_ps")
                    nc.tensor.matmul(qq_ps[:], w2T[:], qT[:], start=True, stop=True)
                    cell["ps"] = qq_ps

                def qq_cp():
                    nc.vector.tensor_scalar_add(
                        out=arena[:, MASKW + b * QS : MASKW + (b + 1) * QS],
                        in0=cell["ps"][:],
                        scalar1=b2[:],
                    )

                return [(qq_mm, qq_cp)]

            def v_tasks(valT, v_ext):
                """Direct-orientation vproj: out[s, c] tiles, batched copies."""
                pairs = []
                # ones column: c'=64 of each slot
                pairs.append(
                    (
                        lambda: nc.vector.memset(
                            v_ext[:].rearrange("p (t c) -> p t c", c=SLOT)[
                                :, :, C : C + 1
                            ],
                            1.0,
                        ),
                        None,
                    )
                )
                # first group small so its copy lands (in program order)
                # before the first va2 pair enters the PE queue
                bounds = [0, 2, 8, 14, 20, 26, 32]
                for gi in range(len(bounds) - 1):
                    g0, g1 = bounds[gi], bounds[gi + 1]
                    gs = g1 - g0
                    cell = {}
                    for k in range(gs):

                        def vp_mm(k=k, g0=g0, cell=cell, first=(k == 0)):
                            if first:
                                cell["ps"] = ppps.tile(
                                    [128, 7 * C], f32, tag="pp", name="vp_ps"
                                )
                            st = g0 + k
                            nc.tensor.matmul(
                                cell["ps"][:, k * C : (k + 1) * C],
                                valT[:, st * 128 : (st + 1) * 128],
                                wvT[:],
                                start=True,
                                stop=True,
                            )

                        pairs.append((vp_mm, None))

                    def vp_cp(g0=g0, gs=gs, cell=cell):
                        nc.vector.tensor_copy(
                            out=v_ext[:, g0 * SLOT : (g0 + gs) * SLOT].rearrange(
                                "p (t c) -> p t c", c=SLOT
                            )[:, :, :C],
                            in_=cell["ps"][:, : gs * C].rearrange(
                                "p (t c) -> p t c", c=C
                            ),
                        )

                    pairs.append((None, vp_cp))
                return pairs

            def stagger(pairs):
                """Each step emits the PREVIOUS task's copy before this task's
                mm so the single-buffer pp ring never stalls the PE queue."""
                steps = []
                prev_cp = [None]

                def mk(mm, pc):
                    def step():
                        if pc is not None:
                            pc()
                        if mm is not None:
                            mm()

                    return step

                for mm, cp in pairs:
                    steps.append(mk(mm, prev_cp[0]))
                    prev_cp[0] = cp
                if prev_cp[0] is not None:
                    steps.append(lambda pc=prev_cp[0]: pc())
                return steps

            def epilogue_tasks(b, acc_ps):
                cell = {}
                steps = []

                def comb_step():
                    comb = work.tile([C + 1, QS], f32, tag="comb")
                    nc.vector.tensor_copy(out=comb[:], in_=acc_ps[:])
                    fin = work.tile([128, QT * C], f32, tag="fin")
                    cell["comb"] = comb
                    cell["fin"] = fin

                steps.append(comb_step)
                for qt in range(QT):

                    def qt_step(qt=qt):
                        ot_ps = ppps.tile([128, C + 1], f32, tag="pp", name="ot_ps")
                        nc.tensor.transpose(
                            ot_ps[:],
                            cell["comb"][:, qt * 128 : (qt + 1) * 128],
                            ident_f[: C + 1, : C + 1],
                        )
                        recip = work.tile([128, 1], f32, tag="recip")
                        nc.vector.reciprocal(recip[:], ot_ps[:, C : C + 1])
                        nc.vector.affine_then_add(
                            out=cell["fin"][:, qt * C : (qt + 1) * C],
                            in0=ot_ps[:, :C],
                            in1=bvrep[:],
                            scale=recip[:],
                            bias=0.0,
                        )

                    steps.append(qt_step)

                def out_step():
                    nc.sync.dma_start(
                        out=out_d[:, b * QT * C : (b + 1) * QT * C],
                        in_=cell["fin"][:],
                    )

                steps.append(out_step)
                return steps

            # ---------- flat pipeline over all B*NG groups ----------
            # Batch 0 prologue runs INLINE (not via pending): qq + all vproj
            # matmuls execute on PE while the mask arena streams in, instead
            # of scores(0,*) blocking the in-order PE queue on the mask DMAs.
            loaded = load_batch(0)
            key8_by_b = {0: loaded[1]}
            vext_by_b = {}
            alpha_by_b = {}
            for mm, cp in qq_tasks(0, loaded[0]):
                mm()
                if cp is not None:
                    cp()
            v_ext0 = pb.tile([128, ST * SLOT], a_dt, tag="v_ext")
            vext_by_b[0] = v_ext0
            load_mask()
            for step in stagger(v_tasks(loaded[2], v_ext0)):
                step()
            pending = []

            total = B * NG
            KEXP = 1  # exp lags scores by 1 group
            scs = {}
            acc_by_b = {}
            va_cursor = 0  # global va2 pair index
            exp_done = -1  # last global group exp'd

            def issue_va2(limit_pairs):
                """Issue ready va2 pairs up to global pair index `limit_pairs`
                (exclusive)."""
                nonlocal va_cursor
                while va_cursor < min(limit_pairs, B * NP):
                    v = va_cursor
                    b_v, u = divmod(v, NP)
                    if u == 0:
                        acc_by_b[b_v] = accps.tile(
                            [C + 1, QS], f32, tag="acc", name="acc"
                        )
                    acc = acc_by_b[b_v]
                    v_ext = vext_by_b[b_v]
                    alpha = alpha_by_b[b_v]
                    if ALPHA_FP8:
                        lhsT = v_ext[:, u * 2 * SLOT : (u + 1) * 2 * SLOT].rearrange(
                            "p (j c) -> p j c", c=SLOT
                        )[:, :, : C + 1]
                        rhs = alpha[:, u * 1024 : (u + 1) * 1024].rearrange(
                            "p (j q) -> p j q", j=2
                        )
                        nc.tensor.matmul(
                            acc[:],
                            lhsT,
                            rhs,
                            start=(u == 0),
                            stop=(u == NP - 1),
                            perf_mode=mybir.MatmulPerfMode.DoubleRow,
                        )
                    else:
                        for h in range(2):
                            st = 2 * u + h
                            nc.tensor.matmul(
                                acc[:],
                                v_ext[:, st * SLOT : st * SLOT + C + 1],
                                alpha[:, st * 512 : (st + 1) * 512],
                                start=(st == 0),
                                stop=(st == ST - 1),
                            )
                    va_cursor += 1
                    if u == NP - 1:
                        ep = epilogue_tasks(b_v, acc_by_b.pop(b_v))
                        pending[:0] = [ep[0]]
                        pending.extend(ep[1:])
                        del vext_by_b[b_v]
                        del alpha_by_b[b_v]

            for gp in range(total + KEXP + 2):
                if gp < total:
                    b_s, g_s = divmod(gp, NG)
                    if g_s == 0:
                        alpha_by_b[b_s] = apool.tile(
                            [128, ST * 512], a_dt, tag="alpha", name="alpha"
                        )
                        if b_s + 1 < B:
                            nxt = load_batch(b_s + 1)
                            key8_by_b[b_s + 1] = nxt[1]
                            v_ext_n = pb.tile([128, ST * SLOT], a_dt, tag="v_ext")
                            vext_by_b[b_s + 1] = v_ext_n
                            pending.extend(
                                stagger(
                                    qq_tasks(b_s + 1, nxt[0])
                                    + v_tasks(nxt[2], v_ext_n)
                                )
                            )
                    key8 = key8_by_b[b_s]
                    w = _gwidth(g_s)
                    sc = scps.tile([128, 1536], f32, tag="sc", name="sc")
                    for ci in range(w):
                        m = _gcol(g_s) + ci
                        lhsT = key8[:, m * 256 : (m + 1) * 256].rearrange(
                            "p (j s) -> p j s", j=2
                        )
                        nc.tensor.matmul(
                            sc[:, ci * 512 : (ci + 1) * 512],
                            lhsT,
                            scores_rhs(b_s, m),
                            start=True,
                            stop=True,
                            perf_mode=DR,
                        )
                    scs[gp] = (sc, b_s, g_s, w)
                if KEXP <= gp < total + KEXP:
                    v = gp - KEXP
                    sc, b_e, g_e, w = scs.pop(v)
                    alpha = alpha_by_b[b_e]
                    c0 = _gcol(g_e) * 512
                    nc.scalar.activation(
                        out=alpha[:, c0 : c0 + w * 512],
                        in_=sc[:, : w * 512],
                        func=AF.Exp,
                        scale=SCALE,
                    )
                    exp_done = v
                    # va2 pairs fully covered by exp'd groups (one group slack)
                    eb, eg = divmod(exp_done, NG)
                    # within batch eb: chunks done through _gcol(eg)+_gwidth(eg)
                    chunks_done = _gcol(eg) + _gwidth(eg)
                    pairs_done = chunks_done // 2
                    issue_va2(eb * NP + max(0, pairs_done - 2))
                if gp >= total + KEXP:
                    issue_va2(B * NP)
                for thresh in (0, 0, 8, 16, 24):
                    if len(pending) > thresh:
                        pending.pop(0)()
            issue_va2(B * NP)
            while pending:
                pending.pop(0)()

    nc.finalize()
    return nc


_nc_cache = None


def kernel(**inputs):
    global _nc_cache, LAST_RESULT
    _install_ntff_hook()
    import ml_dtypes

    from concourse.bass_utils import run_bass_kernel_spmd

    bf16 = ml_dtypes.bfloat16
    f8 = ml_dtypes.float8_e4m3
    arrs = {k: np.asarray(v) for k, v in inputs.items()}

    keyT = np.ascontiguousarray(
        arrs["key"].astype(np.float32).transpose(2, 1, 0)
    )  # [D, B, S]
    key8 = np.zeros([D, B, ST * 256], dtype=f8)
    eye8 = np.eye(128, dtype=np.float32).astype(f8)
    k8 = keyT.astype(f8)
    for st in range(ST):
        key8[:, :, st * 256 : st * 256 + 128] = eye8[:, None, :]
        key8[:, :, st * 256 + 128 : (st + 1) * 256] = k8[
            :, :, st * 128 : (st + 1) * 128
        ]

    value = np.ascontiguousarray(
        arrs["value"].astype(np.float32).transpose(2, 1, 0)
    ).astype(bf16)  # [D, B, S]
    query = np.ascontiguousarray(arrs["query"], dtype=np.float32)
    mask = np.ascontiguousarray(arrs["mask"], dtype=np.int32)
    if mask.ndim == 3:
        mask = mask[0]
    maskbias = np.where(mask == 0, np.float32(MASKBIAS), np.float32(0.0))
    mask8_full = maskbias.astype(f8)  # [S(q-axis? no: rows=q of reference), S]

    wk_f = arrs["wk_w"].astype(np.float64)
    wq_f = arrs["wq_w"].astype(np.float64)
    # scores = (wk key).(wq q + bq) = key^T (W2 q + b2); lhsT for the qq
    # matmul is W2^T = wq^T wk
    w2T = np.ascontiguousarray(wq_f.T @ wk_f).astype(np.float32).astype(bf16)
    b2h = np.ascontiguousarray(
        (wk_f.T @ arrs["wq_b"].astype(np.float64)).astype(np.float32)
    ).reshape(D, 1)
    wvT = np.ascontiguousarray(arrs["wv_w"].astype(np.float32).T).astype(bf16)
    bvrep = np.ascontiguousarray(
        np.broadcast_to(arrs["wv_b"].astype(np.float32)[None, :], (128, C))
    ).astype(np.float32)

    if _nc_cache is None:
        _nc_cache = _build_nc()
    nc = _nc_cache

    in_maps = []
    for i in range(NCORES):
        q0 = i * QS
        # scores^T[s, q] is masked by mask[q_global, s]; arena layout
        # [p, m*512 + q] = maskbias[q0+q, m*128+p]
        mslice = np.ascontiguousarray(
            mask8_full[q0 : q0 + QS, :].T.reshape(ST, 128, QS)
            .transpose(1, 0, 2)
            .reshape(128, MASKW)
        )
        in_maps.append(
            {
                "key8": key8,
                "val": value,
                "query": np.ascontiguousarray(
                    query[q0 : q0 + QS].transpose(2, 1, 0)
                ).astype(bf16),
                "mask8": mslice,
                "w2T": w2T,
                "wvT": wvT,
                "b2": b2h,
                "bvrep": bvrep,
                "vtag": np.zeros([KVER], np.float32),
            }
        )

    trace = bool(int(os.environ.get("KERNEL_TRACE", "0")))
    kw = {}
    if trace:
        kw = dict(trace=True, trace_cores=[0])
    try:
        res = run_bass_kernel_spmd(nc, in_maps, core_ids=list(range(NCORES)), **kw)
    except Exception:
        # transient device wedge (e.g. NRT_EXEC_UNIT_UNRECOVERABLE from an
        # earlier crashed process): one retry after the runtime re-opens
        res = run_bass_kernel_spmd(nc, in_maps, core_ids=list(range(NCORES)), **kw)
    LAST_RESULT = res
    # per-core out is [128, (b, qt, c)]; q_local = qt*128 + p
    cores = []
    for r in res.results:
        oc = r["out"].reshape(128, B, QT, C)
        cores.append(np.ascontiguousarray(oc.transpose(2, 0, 1, 3)).reshape(QS, B, C))
    out = np.concatenate(cores, axis=0)
    return out
